# revision 3
# baseline (speedup 1.0000x reference)
"""TRN2 Bass kernel for nn_CustomLinear: out = x @ weight.T + bias.

Full shapes: x [8192, 4096] f32, weight [4096, 4096] f32, bias [4096] f32,
out [8192, 4096] f32.

Strategy (data-parallel over rows of x, 8 NeuronCores):
  - Each core gets x^T[:, c*1024:(c+1)*1024]  (i.e. 1024 rows of x), full
    weight^T and bias.
  - Per core: out^T_shard[kout, m] = sum_n W^T[n, kout]^T-tiles @ x^T[n, m]
    computed as PE matmuls with the contraction dim (n = in_features) on
    partitions.  x^T shard (16 MB) stays resident in SBUF; W^T (64 MB)
    streams through exactly once.  fp32r matmuls: 1 cycle/row at N=512 with
    ~1.5e-4 rel-l2 accuracy (vs 2.3e-3 for bf16), no cast passes needed.
  - PSUM: 4 accumulation tiles [128, 512] live per kout-group (2 kout tiles
    x 2 m-halves), double-buffered = all 8 banks.
  - Eviction: ScalarE Identity activation adds bias (per-partition = per
    out-feature) while copying PSUM -> SBUF, then DMA to out^T in DRAM.
  - Host side: transpose x/weight into the device layouts, un-transpose the
    gathered out^T shards.
"""

import numpy as np

import concourse.bass as bass
import concourse.mybir as mybir
import concourse.tile as tile
from concourse import bacc
from concourse.bass import ds, ts

M_FULL, IN_F, OUT_F = 8192, 4096, 4096
N_CORES = 8
M_SH = M_FULL // N_CORES  # 1024 rows of x per core
P = 128

_EXEC = None  # lazily built (executor, meta) — compile once per process


def _build_nc(m_sh=M_SH, in_f=IN_F, out_f=OUT_F):
    nt = in_f // P  # contraction tiles
    kt = out_f // P  # out-feature tiles
    kg = 2  # kout tiles per psum group
    ngrp = kt // kg
    nmov = 512  # moving free dim per matmul
    mh = m_sh // nmov  # m chunks per core

    f32 = mybir.dt.float32
    f32r = mybir.dt.float32r

    nc = bacc.Bacc("TRN2", target_bir_lowering=False, debug=False)
    with tile.TileContext(nc) as tc:
        with tc.tile_pool(name="dram", bufs=1, space="DRAM") as dram:
            xt = dram.tile([in_f, m_sh], f32r, kind="ExternalInput", name="xt")
            wt = dram.tile([in_f, out_f], f32r, kind="ExternalInput", name="wt")
            br = dram.tile([P, kt], f32, kind="ExternalInput", name="br")
            ot = dram.tile([out_f, m_sh], f32, kind="ExternalOutput", name="ot")
            names = {
                "xt": xt.tensor.name,
                "wt": wt.tensor.name,
                "br": br.tensor.name,
                "ot": ot.tensor.name,
            }
            with (
                tc.tile_pool(name="const", bufs=1) as const,
                tc.tile_pool(name="xres_pool", bufs=1) as xres_pool,
                tc.tile_pool(name="wpool", bufs=6) as wpool,
                tc.tile_pool(name="psum", bufs=2, space="PSUM") as psum_pool,
                tc.tile_pool(name="opool", bufs=3) as opool,
            ):
                bias_sb = const.tile([P, kt], f32)
                nc.sync.dma_start(out=bias_sb[:], in_=br[:])

                # x^T shard resident in SBUF: nt tiles of [128, m_sh] side
                # by side -> [128, nt*m_sh] (128 KB/partition at full size).
                xres = xres_pool.tile([P, nt * m_sh], f32r)
                for i in range(nt):
                    nc.sync.dma_start(
                        out=xres[:, ts(i, m_sh)], in_=xt[ts(i, P), :]
                    )

                for g in range(ngrp):
                    pss = [
                        [
                            psum_pool.tile(
                                [P, nmov],
                                f32,
                                tag=f"ps_{kk}_{h}",
                                name=f"ps_{g}_{kk}_{h}",
                            )
                            for h in range(mh)
                        ]
                        for kk in range(kg)
                    ]
                    for i in range(nt):
                        wtile = wpool.tile([P, kg * P], f32r, tag="wt")
                        nc.sync.dma_start(
                            out=wtile[:], in_=wt[ts(i, P), ds(g * kg * P, kg * P)]
                        )
                        for kk in range(kg):
                            for h in range(mh):
                                nc.tensor.matmul(
                                    pss[kk][h][:],
                                    wtile[:, ts(kk, P)],
                                    xres[:, ds(i * m_sh + h * nmov, nmov)],
                                    start=(i == 0),
                                    stop=(i == nt - 1),
                                )
                    for kk in range(kg):
                        t = g * kg + kk
                        osb = opool.tile([P, m_sh], f32, tag="osb")
                        for h in range(mh):
                            nc.scalar.activation(
                                osb[:, ts(h, nmov)],
                                pss[kk][h][:],
                                mybir.ActivationFunctionType.Identity,
                                bias=bias_sb[:, ds(t, 1)],
                            )
                        nc.sync.dma_start(out=ot[ts(t, P), :], in_=osb[:])
    nc.compile()
    return nc, names


def _make_executor(nc):
    """Build a once-jitted 8-core shard_map executor for `nc`.

    Mirrors concourse.bass2jax.run_bass_via_pjrt's multi-core path, but the
    jitted function is constructed exactly once so repeat kernel() calls
    don't recompile the NEFF.
    """
    import jax
    from jax.sharding import Mesh, PartitionSpec

    try:
        from jax.experimental.shard_map import shard_map
    except ImportError:  # newer jax
        from jax import shard_map

    from concourse import bass2jax
    from concourse.bass2jax import install_neuronx_cc_hook

    install_neuronx_cc_hook()

    partition_name = (
        nc.partition_id_tensor.name if nc.partition_id_tensor else None
    )
    in_names, out_names, out_avals, zero_outs = [], [], [], []
    for alloc in nc.m.functions[0].allocations:
        if not isinstance(alloc, mybir.MemoryLocationSet):
            continue
        name = alloc.memorylocations[0].name
        if alloc.kind == "ExternalInput":
            if name != partition_name:
                in_names.append(name)
        elif alloc.kind == "ExternalOutput":
            shape = tuple(alloc.tensor_shape)
            dtype = mybir.dt.np(alloc.dtype)
            out_names.append(name)
            out_avals.append(jax.core.ShapedArray(shape, dtype))
            zero_outs.append((shape, dtype))
    n_params = len(in_names)
    all_in_names = list(in_names) + list(out_names)
    if partition_name is not None:
        all_in_names.append(partition_name)

    def _body(*args):
        operands = list(args)
        if partition_name is not None:
            operands.append(bass2jax.partition_id_tensor())
        outs = bass2jax._bass_exec_p.bind(
            *operands,
            out_avals=tuple(out_avals),
            in_names=tuple(all_in_names),
            out_names=tuple(out_names),
            lowering_input_output_aliases=(),
            sim_require_finite=True,
            sim_require_nnan=True,
            nc=nc,
        )
        return tuple(outs)

    devices = jax.devices()[:N_CORES]
    mesh = Mesh(np.asarray(devices), ("core",))
    n_outs = len(out_names)
    in_specs = (PartitionSpec("core"),) * (n_params + n_outs)
    out_specs = (PartitionSpec("core"),) * n_outs
    # No donation: our kernel writes every output element, and undonated
    # zero buffers can be reused across timed calls.
    sharded = jax.jit(
        shard_map(
            _body,
            mesh=mesh,
            in_specs=in_specs,
            out_specs=out_specs,
            check_rep=False,
        ),
        keep_unused=True,
    )
    meta = {
        "in_names": in_names,
        "out_names": out_names,
        "zero_outs": zero_outs,
    }
    return sharded, meta


def _get_exec():
    global _EXEC
    if _EXEC is None:
        nc, names = _build_nc()
        sharded, meta = _make_executor(nc)
        meta["names"] = names
        _EXEC = (sharded, meta)
    return _EXEC


def _prep_inputs(x, weight, bias):
    """Host-side shard prep -> concatenated global arrays (axis 0 = core)."""
    xt = np.ascontiguousarray(x.T)  # [in_f, m_full]
    wt = np.ascontiguousarray(weight.T)  # [in_f, out_f]
    br = np.ascontiguousarray(bias.reshape(OUT_F // P, P).T)  # [P, kt]
    # per-core xt shard is xt[:, c*M_SH:(c+1)*M_SH]; global concat on axis 0
    xt_g = np.concatenate(
        [xt[:, c * M_SH : (c + 1) * M_SH] for c in range(N_CORES)], axis=0
    )
    wt_g = np.concatenate([wt] * N_CORES, axis=0)
    br_g = np.concatenate([br] * N_CORES, axis=0)
    return {"xt": xt_g, "wt": wt_g, "br": br_g}


def _run(concat_inputs):
    sharded, meta = _get_exec()
    names = meta["names"]
    args = [concat_inputs[k] for k in ("xt", "wt", "br")]
    # order must match in_names (allocation order); map by name
    by_name = {
        names["xt"]: concat_inputs["xt"],
        names["wt"]: concat_inputs["wt"],
        names["br"]: concat_inputs["br"],
    }
    args = [by_name[n] for n in meta["in_names"]]
    zeros = [
        np.zeros((N_CORES * s[0], *s[1:]), d) for s, d in meta["zero_outs"]
    ]
    outs = sharded(*args, *zeros)
    return {n: np.asarray(o) for n, o in zip(meta["out_names"], outs)}


def kernel(x: np.ndarray, weight: np.ndarray, bias: np.ndarray) -> np.ndarray:
    x = np.asarray(x, dtype=np.float32)
    weight = np.asarray(weight, dtype=np.float32)
    bias = np.asarray(bias, dtype=np.float32)

    concat = _prep_inputs(x, weight, bias)
    outs = _run(concat)
    _, meta = _get_exec()
    ot_g = outs[meta["names"]["ot"]]  # [N_CORES*out_f, M_SH]
    out = np.empty((M_FULL, OUT_F), dtype=np.float32)
    for c in range(N_CORES):
        out[c * M_SH : (c + 1) * M_SH, :] = ot_g[
            c * OUT_F : (c + 1) * OUT_F, :
        ].T
    return out


# revision 9
# speedup vs baseline: 129.6832x; 129.6832x over previous
"""TRN2 Bass kernel for nn_CustomLinear: out = x @ weight.T + bias.

Full shapes: x [8192, 4096] f32, weight [4096, 4096] f32, bias [4096] f32,
out [8192, 4096] f32.

Strategy (data-parallel over rows of x, 8 NeuronCores):
  - Each core gets x^T[:, c*1024:(c+1)*1024]  (i.e. 1024 rows of x), full
    weight^T and bias.
  - Per core: out^T_shard[kout, m] = sum_n W^T[n, kout]^T-tiles @ x^T[n, m]
    computed as PE matmuls with the contraction dim (n = in_features) on
    partitions.  x^T shard (16 MB) stays resident in SBUF; W^T (64 MB)
    streams through exactly once.  fp32r matmuls: 1 cycle/row at N=512 with
    ~1.5e-4 rel-l2 accuracy (vs 2.3e-3 for bf16), no cast passes needed.
  - PSUM: 4 accumulation tiles [128, 512] live per kout-group (2 kout tiles
    x 2 m-halves), double-buffered = all 8 banks.
  - Eviction: ScalarE Identity activation adds bias (per-partition = per
    out-feature) while copying PSUM -> SBUF, then DMA to out^T in DRAM.
  - Host side: transpose x/weight into the device layouts, un-transpose the
    gathered out^T shards.
"""

import numpy as np

import concourse.bass as bass
import concourse.mybir as mybir
import concourse.tile as tile
from concourse import bacc
from concourse.bass import ds, ts

M_FULL, IN_F, OUT_F = 8192, 4096, 4096
N_CORES = 8
M_SH = M_FULL // N_CORES  # 1024 rows of x per core
P = 128

_EXEC = None  # lazily built (executor, meta) — compile once per process


def _build_nc(m_sh=M_SH, in_f=IN_F, out_f=OUT_F, reps=1):
    nt = in_f // P  # contraction tiles
    kt = out_f // P  # out-feature tiles
    kg = 2  # kout tiles per psum group
    ngrp = kt // kg
    nmov = 512  # moving free dim per matmul
    mh = m_sh // nmov  # m chunks per core

    f32 = mybir.dt.float32
    f32r = mybir.dt.float32r

    nc = bacc.Bacc("TRN2", target_bir_lowering=False, debug=False)
    with tile.TileContext(nc) as tc:
        with tc.tile_pool(name="dram", bufs=1, space="DRAM") as dram:
            xt = dram.tile([in_f, m_sh], f32r, kind="ExternalInput", name="xt")
            wt = dram.tile([in_f, out_f], f32r, kind="ExternalInput", name="wt")
            br = dram.tile([P, kt], f32, kind="ExternalInput", name="br")
            ot = dram.tile([out_f, m_sh], f32, kind="ExternalOutput", name="ot")
            names = {
                "xt": xt.tensor.name,
                "wt": wt.tensor.name,
                "br": br.tensor.name,
                "ot": ot.tensor.name,
            }
            with (
                tc.tile_pool(name="const", bufs=1) as const,
                tc.tile_pool(name="xres_pool", bufs=1) as xres_pool,
                tc.tile_pool(name="wpool", bufs=6) as wpool,
                tc.tile_pool(name="psum", bufs=2, space="PSUM") as psum_pool,
                tc.tile_pool(name="opool", bufs=3) as opool,
            ):
                bias_sb = const.tile([P, kt], f32)
                nc.sync.dma_start(out=bias_sb[:], in_=br[:])

                # x^T shard resident in SBUF: nt tiles of [128, m_sh] side
                # by side -> [128, nt*m_sh] (128 KB/partition at full size).
                xres = xres_pool.tile([P, nt * m_sh], f32r)
                for rep in range(reps):
                  for i in range(nt):
                    nc.sync.dma_start(
                        out=xres[:, ts(i, m_sh)], in_=xt[ts(i, P), :]
                    )

                  for g in range(ngrp):
                    pss = [
                        [
                            psum_pool.tile(
                                [P, nmov],
                                f32,
                                tag=f"ps_{kk}_{h}",
                                name=f"ps_{g}_{kk}_{h}",
                            )
                            for h in range(mh)
                        ]
                        for kk in range(kg)
                    ]
                    for i in range(nt):
                        wtile = wpool.tile([P, kg * P], f32r, tag="wt")
                        nc.sync.dma_start(
                            out=wtile[:], in_=wt[ts(i, P), ds(g * kg * P, kg * P)]
                        )
                        for kk in range(kg):
                            for h in range(mh):
                                nc.tensor.matmul(
                                    pss[kk][h][:],
                                    wtile[:, ts(kk, P)],
                                    xres[:, ds(i * m_sh + h * nmov, nmov)],
                                    start=(i == 0),
                                    stop=(i == nt - 1),
                                )
                    for kk in range(kg):
                        t = g * kg + kk
                        osb = opool.tile([P, m_sh], f32, tag="osb")
                        for h in range(mh):
                            nc.scalar.activation(
                                osb[:, ts(h, nmov)],
                                pss[kk][h][:],
                                mybir.ActivationFunctionType.Identity,
                                bias=bias_sb[:, ds(t, 1)],
                            )
                        nc.sync.dma_start(out=ot[ts(t, P), :], in_=osb[:])
    nc.compile()
    return nc, names


def _make_executor(nc, replicated_names=()):
    """Build a once-jitted 8-core shard_map executor for `nc`.

    Mirrors concourse.bass2jax.run_bass_via_pjrt's multi-core path, but the
    jitted function is constructed exactly once so repeat kernel() calls
    don't recompile the NEFF, and inputs listed in `replicated_names` are
    passed once and replicated to all cores (instead of 8x host concat).
    """
    import jax
    from jax.sharding import Mesh, PartitionSpec

    try:
        from jax.experimental.shard_map import shard_map
    except ImportError:  # newer jax
        from jax import shard_map

    from concourse import bass2jax
    from concourse.bass2jax import install_neuronx_cc_hook

    install_neuronx_cc_hook()

    partition_name = (
        nc.partition_id_tensor.name if nc.partition_id_tensor else None
    )
    replicated = set(replicated_names or ())
    in_names, out_names, out_avals, zero_outs = [], [], [], []
    for alloc in nc.m.functions[0].allocations:
        if not isinstance(alloc, mybir.MemoryLocationSet):
            continue
        name = alloc.memorylocations[0].name
        if alloc.kind == "ExternalInput":
            if name != partition_name:
                in_names.append(name)
        elif alloc.kind == "ExternalOutput":
            shape = tuple(alloc.tensor_shape)
            dtype = mybir.dt.np(alloc.dtype)
            out_names.append(name)
            out_avals.append(jax.core.ShapedArray(shape, dtype))
            zero_outs.append((shape, dtype))
    n_params = len(in_names)
    all_in_names = list(in_names) + list(out_names)
    if partition_name is not None:
        all_in_names.append(partition_name)

    def _body(*args):
        operands = list(args)
        if partition_name is not None:
            operands.append(bass2jax.partition_id_tensor())
        outs = bass2jax._bass_exec_p.bind(
            *operands,
            out_avals=tuple(out_avals),
            in_names=tuple(all_in_names),
            out_names=tuple(out_names),
            lowering_input_output_aliases=(),
            sim_require_finite=True,
            sim_require_nnan=True,
            nc=nc,
        )
        return tuple(outs)

    devices = jax.devices()[:N_CORES]
    mesh = Mesh(np.asarray(devices), ("core",))
    n_outs = len(out_names)
    in_specs = tuple(
        PartitionSpec() if n in replicated else PartitionSpec("core")
        for n in in_names
    ) + (PartitionSpec("core"),) * n_outs
    out_specs = (PartitionSpec("core"),) * n_outs
    # No donation: our kernel writes every output element, and undonated
    # zero buffers can be reused across timed calls.
    sharded = jax.jit(
        shard_map(
            _body,
            mesh=mesh,
            in_specs=in_specs,
            out_specs=out_specs,
            check_rep=False,
        ),
        keep_unused=True,
    )
    meta = {
        "in_names": in_names,
        "out_names": out_names,
        "zero_outs": zero_outs,
        "in_specs": in_specs,
        "mesh": mesh,
    }
    return sharded, meta


def _get_exec():
    global _EXEC
    if _EXEC is None:
        nc, names = _build_nc()
        sharded, meta = _make_executor(
            nc, replicated_names=(names["wt"], names["br"])
        )
        meta["names"] = names
        _EXEC = (sharded, meta)
    return _EXEC


def _prep_inputs(x, weight, bias):
    """Host-side prep into device layouts.

    xt: per-core x^T shards stacked on axis 0 -> [N_CORES*in_f, m_sh]
    wt: weight^T [in_f, out_f] (replicated by shard_map)
    br: bias grouped per out-feature tile [P, kt] (replicated)
    """
    xt_g = np.ascontiguousarray(
        x.reshape(N_CORES, M_SH, IN_F).transpose(0, 2, 1)
    ).reshape(N_CORES * IN_F, M_SH)
    wt = np.ascontiguousarray(weight.T)
    br = np.ascontiguousarray(bias.reshape(OUT_F // P, P).T)
    return {"xt": xt_g, "wt": wt, "br": br}


def _make_args(prepped, meta):
    """Ordered positional args (+ fresh zero output buffers) for the jitted
    executor."""
    names = meta["names"]
    by_name = {
        names["xt"]: prepped["xt"],
        names["wt"]: prepped["wt"],
        names["br"]: prepped["br"],
    }
    args = [by_name[n] for n in meta["in_names"]]
    zeros = [
        np.zeros((N_CORES * s[0], *s[1:]), d) for s, d in meta["zero_outs"]
    ]
    return args, zeros


def kernel(x: np.ndarray, weight: np.ndarray, bias: np.ndarray) -> np.ndarray:
    x = np.asarray(x, dtype=np.float32)
    weight = np.asarray(weight, dtype=np.float32)
    bias = np.asarray(bias, dtype=np.float32)

    sharded, meta = _get_exec()
    args, zeros = _make_args(_prep_inputs(x, weight, bias), meta)
    outs = sharded(*args, *zeros)
    out_by_name = {
        n: np.asarray(o) for n, o in zip(meta["out_names"], outs)
    }
    ot_g = out_by_name[meta["names"]["ot"]]  # [N_CORES*out_f, M_SH]
    out = np.ascontiguousarray(
        ot_g.reshape(N_CORES, OUT_F, M_SH).transpose(0, 2, 1)
    ).reshape(M_FULL, OUT_F)
    return out


# revision 11
# speedup vs baseline: 169.0000x; 1.3032x over previous
"""TRN2 Bass kernel for nn_CustomLinear: out = x @ weight.T + bias.

Full shapes: x [8192, 4096] f32, weight [4096, 4096] f32, bias [4096] f32,
out [8192, 4096] f32.

Strategy (data-parallel over rows of x, 8 NeuronCores):
  - Each core gets x^T[:, c*1024:(c+1)*1024]  (i.e. 1024 rows of x), full
    weight^T and bias.
  - Per core: out^T_shard[kout, m] = sum_n W^T[n, kout]^T-tiles @ x^T[n, m]
    computed as PE matmuls with the contraction dim (n = in_features) on
    partitions.  x^T shard (16 MB) stays resident in SBUF; W^T (64 MB)
    streams through exactly once.  fp32r matmuls: 1 cycle/row at N=512 with
    ~1.5e-4 rel-l2 accuracy (vs 2.3e-3 for bf16), no cast passes needed.
  - PSUM: 4 accumulation tiles [128, 512] live per kout-group (2 kout tiles
    x 2 m-halves), double-buffered = all 8 banks.
  - Eviction: ScalarE Identity activation adds bias (per-partition = per
    out-feature) while copying PSUM -> SBUF, then DMA to out^T in DRAM.
  - Host side: transpose x/weight into the device layouts, un-transpose the
    gathered out^T shards.
"""

import numpy as np

import concourse.bass as bass
import concourse.mybir as mybir
import concourse.tile as tile
from concourse import bacc
from concourse.bass import ds, ts

M_FULL, IN_F, OUT_F = 8192, 4096, 4096
N_CORES = 8
M_SH = M_FULL // N_CORES  # 1024 rows of x per core
P = 128

_EXEC = None  # lazily built (executor, meta) — compile once per process


def _build_nc(m_sh=M_SH, in_f=IN_F, out_f=OUT_F, reps=1):
    nt = in_f // P  # contraction tiles
    kt = out_f // P  # out-feature tiles
    kg = 2  # kout tiles per psum group
    ngrp = kt // kg
    nmov = 512  # moving free dim per matmul
    mh = m_sh // nmov  # m chunks per core

    f32 = mybir.dt.float32
    f32r = mybir.dt.float32r

    nc = bacc.Bacc("TRN2", target_bir_lowering=False, debug=False)
    with tile.TileContext(nc) as tc:
        with tc.tile_pool(name="dram", bufs=1, space="DRAM") as dram:
            xt = dram.tile([in_f, m_sh], f32r, kind="ExternalInput", name="xt")
            wt = dram.tile([in_f, out_f], f32r, kind="ExternalInput", name="wt")
            br = dram.tile([P, kt], f32, kind="ExternalInput", name="br")
            ot = dram.tile([out_f, m_sh], f32, kind="ExternalOutput", name="ot")
            names = {
                "xt": xt.tensor.name,
                "wt": wt.tensor.name,
                "br": br.tensor.name,
                "ot": ot.tensor.name,
            }
            with (
                tc.tile_pool(name="const", bufs=1) as const,
                tc.tile_pool(name="xres_pool", bufs=1) as xres_pool,
                tc.tile_pool(name="wpool", bufs=6) as wpool,
                tc.tile_pool(name="psum", bufs=2, space="PSUM") as psum_pool,
                tc.tile_pool(name="opool", bufs=3) as opool,
            ):
                bias_sb = const.tile([P, kt], f32)
                nc.sync.dma_start(out=bias_sb[:], in_=br[:])

                # x^T shard resident in SBUF: nt tiles of [128, m_sh] side
                # by side -> [128, nt*m_sh] (128 KB/partition at full size).
                xres = xres_pool.tile([P, nt * m_sh], f32r)
                for rep in range(reps):
                  for g in range(ngrp):
                    pss = [
                        [
                            psum_pool.tile(
                                [P, nmov],
                                f32,
                                tag=f"ps_{kk}_{h}",
                                name=f"ps_{g}_{kk}_{h}",
                            )
                            for h in range(mh)
                        ]
                        for kk in range(kg)
                    ]
                    for i in range(nt):
                        if g == 0:
                            # interleave the x^T residency load with group
                            # 0's W stream so neither queues behind the
                            # other (saves ~14 us vs loading all of x
                            # up front, per the cost model)
                            nc.sync.dma_start(
                                out=xres[:, ts(i, m_sh)], in_=xt[ts(i, P), :]
                            )
                        wtile = wpool.tile([P, kg * P], f32r, tag="wt")
                        nc.sync.dma_start(
                            out=wtile[:], in_=wt[ts(i, P), ds(g * kg * P, kg * P)]
                        )
                        for kk in range(kg):
                            for h in range(mh):
                                nc.tensor.matmul(
                                    pss[kk][h][:],
                                    wtile[:, ts(kk, P)],
                                    xres[:, ds(i * m_sh + h * nmov, nmov)],
                                    start=(i == 0),
                                    stop=(i == nt - 1),
                                )
                    for kk in range(kg):
                        t = g * kg + kk
                        osb = opool.tile([P, m_sh], f32, tag="osb")
                        for h in range(mh):
                            nc.scalar.activation(
                                osb[:, ts(h, nmov)],
                                pss[kk][h][:],
                                mybir.ActivationFunctionType.Identity,
                                bias=bias_sb[:, ds(t, 1)],
                            )
                        nc.sync.dma_start(out=ot[ts(t, P), :], in_=osb[:])
    nc.compile()
    return nc, names


def _make_executor(nc, replicated_names=()):
    """Build a once-jitted 8-core shard_map executor for `nc`.

    Mirrors concourse.bass2jax.run_bass_via_pjrt's multi-core path, but the
    jitted function is constructed exactly once so repeat kernel() calls
    don't recompile the NEFF, and inputs listed in `replicated_names` are
    passed once and replicated to all cores (instead of 8x host concat).
    """
    import jax
    from jax.sharding import Mesh, PartitionSpec

    try:
        from jax.experimental.shard_map import shard_map
    except ImportError:  # newer jax
        from jax import shard_map

    from concourse import bass2jax
    from concourse.bass2jax import install_neuronx_cc_hook

    install_neuronx_cc_hook()

    partition_name = (
        nc.partition_id_tensor.name if nc.partition_id_tensor else None
    )
    replicated = set(replicated_names or ())
    in_names, out_names, out_avals, zero_outs = [], [], [], []
    for alloc in nc.m.functions[0].allocations:
        if not isinstance(alloc, mybir.MemoryLocationSet):
            continue
        name = alloc.memorylocations[0].name
        if alloc.kind == "ExternalInput":
            if name != partition_name:
                in_names.append(name)
        elif alloc.kind == "ExternalOutput":
            shape = tuple(alloc.tensor_shape)
            dtype = mybir.dt.np(alloc.dtype)
            out_names.append(name)
            out_avals.append(jax.core.ShapedArray(shape, dtype))
            zero_outs.append((shape, dtype))
    n_params = len(in_names)
    all_in_names = list(in_names) + list(out_names)
    if partition_name is not None:
        all_in_names.append(partition_name)

    def _body(*args):
        operands = list(args)
        if partition_name is not None:
            operands.append(bass2jax.partition_id_tensor())
        outs = bass2jax._bass_exec_p.bind(
            *operands,
            out_avals=tuple(out_avals),
            in_names=tuple(all_in_names),
            out_names=tuple(out_names),
            lowering_input_output_aliases=(),
            sim_require_finite=True,
            sim_require_nnan=True,
            nc=nc,
        )
        return tuple(outs)

    devices = jax.devices()[:N_CORES]
    mesh = Mesh(np.asarray(devices), ("core",))
    n_outs = len(out_names)
    in_specs = tuple(
        PartitionSpec() if n in replicated else PartitionSpec("core")
        for n in in_names
    ) + (PartitionSpec("core"),) * n_outs
    out_specs = (PartitionSpec("core"),) * n_outs
    # No donation: our kernel writes every output element, and undonated
    # zero buffers can be reused across timed calls.
    sharded = jax.jit(
        shard_map(
            _body,
            mesh=mesh,
            in_specs=in_specs,
            out_specs=out_specs,
            check_rep=False,
        ),
        keep_unused=True,
    )
    meta = {
        "in_names": in_names,
        "out_names": out_names,
        "zero_outs": zero_outs,
        "in_specs": in_specs,
        "mesh": mesh,
    }
    return sharded, meta


def _get_exec():
    global _EXEC
    if _EXEC is None:
        nc, names = _build_nc()
        sharded, meta = _make_executor(
            nc, replicated_names=(names["wt"], names["br"])
        )
        meta["names"] = names
        _EXEC = (sharded, meta)
    return _EXEC


def _prep_inputs(x, weight, bias):
    """Host-side prep into device layouts.

    xt: per-core x^T shards stacked on axis 0 -> [N_CORES*in_f, m_sh]
    wt: weight^T [in_f, out_f] (replicated by shard_map)
    br: bias grouped per out-feature tile [P, kt] (replicated)
    """
    xt_g = np.ascontiguousarray(
        x.reshape(N_CORES, M_SH, IN_F).transpose(0, 2, 1)
    ).reshape(N_CORES * IN_F, M_SH)
    wt = np.ascontiguousarray(weight.T)
    br = np.ascontiguousarray(bias.reshape(OUT_F // P, P).T)
    return {"xt": xt_g, "wt": wt, "br": br}


def _make_args(prepped, meta):
    """Ordered positional args (+ fresh zero output buffers) for the jitted
    executor."""
    names = meta["names"]
    by_name = {
        names["xt"]: prepped["xt"],
        names["wt"]: prepped["wt"],
        names["br"]: prepped["br"],
    }
    args = [by_name[n] for n in meta["in_names"]]
    zeros = [
        np.zeros((N_CORES * s[0], *s[1:]), d) for s, d in meta["zero_outs"]
    ]
    return args, zeros


def kernel(x: np.ndarray, weight: np.ndarray, bias: np.ndarray) -> np.ndarray:
    x = np.asarray(x, dtype=np.float32)
    weight = np.asarray(weight, dtype=np.float32)
    bias = np.asarray(bias, dtype=np.float32)

    sharded, meta = _get_exec()
    args, zeros = _make_args(_prep_inputs(x, weight, bias), meta)
    outs = sharded(*args, *zeros)
    out_by_name = {
        n: np.asarray(o) for n, o in zip(meta["out_names"], outs)
    }
    ot_g = out_by_name[meta["names"]["ot"]]  # [N_CORES*out_f, M_SH]
    out = np.ascontiguousarray(
        ot_g.reshape(N_CORES, OUT_F, M_SH).transpose(0, 2, 1)
    ).reshape(M_FULL, OUT_F)
    return out


# revision 12
# speedup vs baseline: 195.5740x; 1.1572x over previous
"""TRN2 Bass kernel for nn_CustomLinear: out = x @ weight.T + bias.

Full shapes: x [8192, 4096] f32, weight [4096, 4096] f32, bias [4096] f32,
out [8192, 4096] f32.

Strategy (data-parallel over rows of x, 8 NeuronCores):
  - Each core gets x^T[:, c*1024:(c+1)*1024]  (i.e. 1024 rows of x), full
    weight^T and bias.
  - Per core: out^T_shard[kout, m] = sum_n W^T[n, kout]^T-tiles @ x^T[n, m]
    computed as PE matmuls with the contraction dim (n = in_features) on
    partitions.  x^T shard (16 MB) stays resident in SBUF; W^T (64 MB)
    streams through exactly once.  fp32r matmuls: 1 cycle/row at N=512 with
    ~1.5e-4 rel-l2 accuracy (vs 2.3e-3 for bf16), no cast passes needed.
  - PSUM: 4 accumulation tiles [128, 512] live per kout-group (2 kout tiles
    x 2 m-halves), double-buffered = all 8 banks.
  - Eviction: ScalarE Identity activation adds bias (per-partition = per
    out-feature) while copying PSUM -> SBUF, then DMA to out^T in DRAM.
  - Host side: transpose x/weight into the device layouts, un-transpose the
    gathered out^T shards.
"""

import numpy as np

import concourse.bass as bass
import concourse.mybir as mybir
import concourse.tile as tile
from concourse import bacc
from concourse.bass import ds, ts

M_FULL, IN_F, OUT_F = 8192, 4096, 4096
N_CORES = 8
M_SH = M_FULL // N_CORES  # 1024 rows of x per core
P = 128

_EXEC = None  # lazily built (executor, meta) — compile once per process


def _build_nc(m_sh=M_SH, in_f=IN_F, out_f=OUT_F, reps=1):
    nt = in_f // P  # contraction tiles
    kt = out_f // P  # out-feature tiles
    kg = 2  # kout tiles per psum group
    ngrp = kt // kg
    nmov = 512  # moving free dim per matmul
    mh = m_sh // nmov  # m chunks per core

    f32 = mybir.dt.float32
    f32r = mybir.dt.float32r

    nc = bacc.Bacc("TRN2", target_bir_lowering=False, debug=False)
    with tile.TileContext(nc) as tc:
        with tc.tile_pool(name="dram", bufs=1, space="DRAM") as dram:
            xt = dram.tile([in_f, m_sh], f32r, kind="ExternalInput", name="xt")
            wt = dram.tile([in_f, out_f], f32r, kind="ExternalInput", name="wt")
            br = dram.tile([P, kt], f32, kind="ExternalInput", name="br")
            ot = dram.tile([out_f, m_sh], f32, kind="ExternalOutput", name="ot")
            names = {
                "xt": xt.tensor.name,
                "wt": wt.tensor.name,
                "br": br.tensor.name,
                "ot": ot.tensor.name,
            }
            with (
                tc.tile_pool(name="const", bufs=1) as const,
                tc.tile_pool(name="xres_pool", bufs=1) as xres_pool,
                tc.tile_pool(name="wpool", bufs=6) as wpool,
                tc.tile_pool(name="psum", bufs=2, space="PSUM") as psum_pool,
                tc.tile_pool(name="opool", bufs=3) as opool,
            ):
                bias_sb = const.tile([P, kt], f32)
                nc.sync.dma_start(out=bias_sb[:], in_=br[:])

                # x^T shard resident in SBUF: nt tiles of [128, m_sh] side
                # by side -> [128, nt*m_sh] (128 KB/partition at full size).
                xres = xres_pool.tile([P, nt * m_sh], f32r)
                for rep in range(reps):
                  for g in range(ngrp):
                    pss = [
                        [
                            psum_pool.tile(
                                [P, nmov],
                                f32,
                                tag=f"ps_{kk}_{h}",
                                name=f"ps_{g}_{kk}_{h}",
                            )
                            for h in range(mh)
                        ]
                        for kk in range(kg)
                    ]
                    for i in range(nt):
                        if g == 0:
                            # interleave the x^T residency load with group
                            # 0's W stream so neither queues behind the
                            # other (saves ~14 us vs loading all of x
                            # up front, per the cost model)
                            nc.sync.dma_start(
                                out=xres[:, ts(i, m_sh)], in_=xt[ts(i, P), :]
                            )
                        wtile = wpool.tile([P, kg * P], f32r, tag="wt")
                        nc.sync.dma_start(
                            out=wtile[:], in_=wt[ts(i, P), ds(g * kg * P, kg * P)]
                        )
                        for kk in range(kg):
                            for h in range(mh):
                                nc.tensor.matmul(
                                    pss[kk][h][:],
                                    wtile[:, ts(kk, P)],
                                    xres[:, ds(i * m_sh + h * nmov, nmov)],
                                    start=(i == 0),
                                    stop=(i == nt - 1),
                                )
                    for kk in range(kg):
                        t = g * kg + kk
                        osb = opool.tile([P, m_sh], f32, tag="osb")
                        for h in range(mh):
                            nc.scalar.activation(
                                osb[:, ts(h, nmov)],
                                pss[kk][h][:],
                                mybir.ActivationFunctionType.Identity,
                                bias=bias_sb[:, ds(t, 1)],
                            )
                        nc.sync.dma_start(out=ot[ts(t, P), :], in_=osb[:])
    nc.compile()
    return nc, names


def _make_executor(nc, replicated_names=()):
    """Build a once-jitted 8-core shard_map executor for `nc`.

    Mirrors concourse.bass2jax.run_bass_via_pjrt's multi-core path, but the
    jitted function is constructed exactly once so repeat kernel() calls
    don't recompile the NEFF, and inputs listed in `replicated_names` are
    passed once and replicated to all cores (instead of 8x host concat).
    """
    import jax
    from jax.sharding import Mesh, PartitionSpec

    try:
        from jax.experimental.shard_map import shard_map
    except ImportError:  # newer jax
        from jax import shard_map

    from concourse import bass2jax
    from concourse.bass2jax import install_neuronx_cc_hook

    install_neuronx_cc_hook()

    partition_name = (
        nc.partition_id_tensor.name if nc.partition_id_tensor else None
    )
    replicated = set(replicated_names or ())
    in_names, out_names, out_avals, zero_outs = [], [], [], []
    for alloc in nc.m.functions[0].allocations:
        if not isinstance(alloc, mybir.MemoryLocationSet):
            continue
        name = alloc.memorylocations[0].name
        if alloc.kind == "ExternalInput":
            if name != partition_name:
                in_names.append(name)
        elif alloc.kind == "ExternalOutput":
            shape = tuple(alloc.tensor_shape)
            dtype = mybir.dt.np(alloc.dtype)
            out_names.append(name)
            out_avals.append(jax.core.ShapedArray(shape, dtype))
            zero_outs.append((shape, dtype))
    n_params = len(in_names)
    all_in_names = list(in_names) + list(out_names)
    if partition_name is not None:
        all_in_names.append(partition_name)

    def _body(*args):
        operands = list(args)
        if partition_name is not None:
            operands.append(bass2jax.partition_id_tensor())
        outs = bass2jax._bass_exec_p.bind(
            *operands,
            out_avals=tuple(out_avals),
            in_names=tuple(all_in_names),
            out_names=tuple(out_names),
            lowering_input_output_aliases=(),
            sim_require_finite=True,
            sim_require_nnan=True,
            nc=nc,
        )
        return tuple(outs)

    devices = jax.devices()[:N_CORES]
    mesh = Mesh(np.asarray(devices), ("core",))
    n_outs = len(out_names)
    in_specs = tuple(
        PartitionSpec() if n in replicated else PartitionSpec("core")
        for n in in_names
    ) + (PartitionSpec("core"),) * n_outs
    out_specs = (PartitionSpec("core"),) * n_outs
    # No donation: our kernel writes every output element, and undonated
    # zero buffers can be reused across timed calls.
    sharded = jax.jit(
        shard_map(
            _body,
            mesh=mesh,
            in_specs=in_specs,
            out_specs=out_specs,
            check_rep=False,
        ),
        keep_unused=True,
    )
    meta = {
        "in_names": in_names,
        "out_names": out_names,
        "zero_outs": zero_outs,
        "in_specs": in_specs,
        "mesh": mesh,
    }
    return sharded, meta


def _get_exec():
    global _EXEC
    if _EXEC is None:
        nc, names = _build_nc()
        sharded, meta = _make_executor(
            nc, replicated_names=(names["wt"], names["br"])
        )
        meta["names"] = names
        _EXEC = (sharded, meta)
    return _EXEC


def _prep_inputs(x, weight, bias):
    """Host-side prep into device layouts.

    xt: per-core x^T shards stacked on axis 0 -> [N_CORES*in_f, m_sh]
    wt: weight^T [in_f, out_f] (replicated by shard_map)
    br: bias grouped per out-feature tile [P, kt] (replicated)
    """
    xt_g = np.ascontiguousarray(
        x.reshape(N_CORES, M_SH, IN_F).transpose(0, 2, 1)
    ).reshape(N_CORES * IN_F, M_SH)
    wt = np.ascontiguousarray(weight.T)
    br = np.ascontiguousarray(bias.reshape(OUT_F // P, P).T)
    return {"xt": xt_g, "wt": wt, "br": br}


def _make_args(prepped, meta):
    """Ordered positional args (+ fresh zero output buffers) for the jitted
    executor."""
    names = meta["names"]
    by_name = {
        names["xt"]: prepped["xt"],
        names["wt"]: prepped["wt"],
        names["br"]: prepped["br"],
    }
    args = [by_name[n] for n in meta["in_names"]]
    zeros = [
        np.zeros((N_CORES * s[0], *s[1:]), d) for s, d in meta["zero_outs"]
    ]
    return args, zeros


def _reset_backend():
    """Best-effort recovery from a wedged NeuronCore: drop the PJRT client
    and jit caches so the next executor build re-handshakes with the (by
    then recovered) axon terminal."""
    global _EXEC
    _EXEC = None
    import jax

    try:
        jax.clear_caches()
    except Exception:
        pass
    try:
        import jax.extend.backend as jeb

        jeb.clear_backends()
    except Exception:
        pass


def kernel(x: np.ndarray, weight: np.ndarray, bias: np.ndarray) -> np.ndarray:
    x = np.asarray(x, dtype=np.float32)
    weight = np.asarray(weight, dtype=np.float32)
    bias = np.asarray(bias, dtype=np.float32)
    prepped = _prep_inputs(x, weight, bias)

    last_exc = None
    for attempt in range(3):
        try:
            sharded, meta = _get_exec()
            args, zeros = _make_args(prepped, meta)
            outs = sharded(*args, *zeros)
            out_by_name = {
                n: np.asarray(o) for n, o in zip(meta["out_names"], outs)
            }
            ot_g = out_by_name[meta["names"]["ot"]]  # [N_CORES*out_f, M_SH]
            return np.ascontiguousarray(
                ot_g.reshape(N_CORES, OUT_F, M_SH).transpose(0, 2, 1)
            ).reshape(M_FULL, OUT_F)
        except Exception as e:  # transient device wedge (NRT unrecoverable)
            last_exc = e
            import time as _time

            _time.sleep(15.0 * (attempt + 1))
            _reset_backend()
    raise last_exc
